# revision 35
# baseline (speedup 1.0000x reference)
"""Multi-head attention (B=1, S=4096, D=512, H=8, HD=64) on 8 trn2 NeuronCores.

Sharding: one head per core (tensor-parallel over heads). Each core computes
its head's Q/K/V projections, flash-style attention entirely on-chip
(transposed layout: scores^T = K Q^T with t on partitions, softmax denominator
via a ones-column folded into the V stationary tiles), applies the output
projection for its head, and writes a full [S, D] partial. The host sums the
8 partials.

v2 layout/engine plan:
- exp work is split between the Activation engine (direct PSUM->SBUF exp with
  fp8e4 output) and GpSimd (CBASE**s via tensor_tensor pow after a DVE
  PSUM->SBUF copy; CBASE = e^SCALE so no separate scale pass is needed).
- E*V runs as fp8e4 DoubleRow matmuls (two per t-tile pair: cols 0:32+ones
  -> psum A rows 0:33 with Z at row 32, cols 32:64 -> psum B rows 0:32).
- Output projection is two accumulating K=32 fp32r matmuls (lo/hi halves).
- V is projected as V^T (full-rate N=512 fp32r) then PE-transposed into
  natural [t, hd] tiles for the DoubleRow stationary operand.
- Chunk 0 of the attention main loop is woven into phase B so the x^T DMA
  and projection head overlaps real work.

Numerics: softmax skips max-subtraction (scores are O(1), exp cannot
overflow); E and V in fp8e4 keeps total rel err ~1.75e-2 < 2e-2.
"""

import numpy as np

import concourse.bacc as bacc
import concourse.mybir as mybir
import concourse.tile as tile
from concourse.bass_utils import run_bass_kernel_spmd

S = 4096          # sequence length
D = 512           # model dim
HD = 64           # head dim
H = 8             # heads == cores
SCALE = HD ** -0.5
CBASE = float(np.exp(SCALE))
P = 128           # partitions
KT = D // P       # 4 k-tiles over the model dim
NSC = S // 512    # 8 s-chunks of 512
NTT = S // P      # 32 t-tiles of 128
NPR = NTT // 2    # 16 t-tile pairs
NST = S // P      # 32 s-tiles of 128

F32 = mybir.dt.float32
F32R = mybir.dt.float32r
F8 = mybir.dt.float8e4

# per-chunk exp path per pair: 'A' = Activation exp, 'G' = DVE copy + GpSimd pow
# pattern[0] must be 'A' (it opens the PSUM accumulation groups).
DEFAULT_PATTERN = "AGAGAGAGAGAGAGAA"
# ys-scale engine per s-tile within a chunk: 'a' = Activation, 'd' = DVE
DEFAULT_MULS = "adda"


def r(ap):
    """fp32 AP -> float32r view (same bits, full-rate PE matmul)."""
    return ap.bitcast(F32R)


def build_kernel(pattern=DEFAULT_PATTERN, muls=DEFAULT_MULS, **_ignored):
    assert len(pattern) == NPR and set(pattern) <= {"A", "G"}
    assert len(muls) == 4 and set(muls) <= {"a", "d"}

    nc = bacc.Bacc(
        "TRN2",
        target_bir_lowering=False,
        debug=False,
        enable_asserts=False,
        num_devices=H,
    )

    xt = nc.dram_tensor("xt", [D, S], F32, kind="ExternalInput").ap()
    wq = nc.dram_tensor("wq", [D, HD], F32, kind="ExternalInput").ap()
    wk = nc.dram_tensor("wk", [D, HD], F32, kind="ExternalInput").ap()
    wv = nc.dram_tensor("wv", [D, HD], F32, kind="ExternalInput").ap()
    wp = nc.dram_tensor("wp", [HD, D], F32, kind="ExternalInput").ap()
    eye = nc.dram_tensor("eye", [HD, HD], F32, kind="ExternalInput").ap()
    y = nc.dram_tensor("y", [S, D], F32, kind="ExternalOutput").ap()

    Exp = mybir.ActivationFunctionType.Exp
    DR = mybir.MatmulPerfMode.DoubleRow
    Pow = mybir.AluOpType.pow

    with tile.TileContext(nc) as tc:
        with (
            tc.tile_pool(name="const", bufs=1) as cp,
            tc.tile_pool(name="exp", bufs=10) as ep,
            tc.tile_pool(name="gsc", bufs=4) as gp,
            tc.tile_pool(name="spsum", bufs=3, space="PSUM") as sp,
            tc.tile_pool(name="opsum", bufs=1, space="PSUM") as op,
        ):
            # ---- persistent SBUF tensors ----
            wq_sb = cp.tile([P, KT, HD], F32, tag="wq")
            wk_sb = cp.tile([P, KT, HD], F32, tag="wk")
            wv_sb = cp.tile([P, KT, HD], F32, tag="wv")
            wpl_sb = cp.tile([32, D], F32, tag="wpl")    # wp rows 0:32
            wph_sb = cp.tile([32, D], F32, tag="wph")    # wp rows 32:64
            qk_sb = cp.tile([HD, 2, S], F32, tag="qk")   # Q^T / K^T merged
            # fp8 V stationary tiles: per pair/parity [V[:,0:32] | ones | pad
            # | V[:,32:64] at 48:80 | pad to 96] (pump stride 96, %16 == 0)
            v8_sb = cp.tile([P, NPR, 2, 96], F8, tag="v8")
            o_lo = cp.tile([33, S], F32, tag="olo")      # O'^T rows hd 0:32 + Z
            o_hi = cp.tile([32, S], F32, tag="ohi")      # O'^T rows hd 32:64
            ones_sb = cp.tile([33, 1], F32, tag="ones")  # row 32 used for Z^T
            rz_sb = cp.tile([P, NST], F32, tag="rz")     # 1/Z, s-tile major
            ebase_sb = cp.tile([P, 2, 512], F32, tag="ebase")  # CBASE const
            id_sb = cp.tile([HD, HD], F32, tag="eye")    # PE-transpose identity

            # ---- inits (DMAs are sequenced inside phase B so the head-
            # critical loads go first on the serialized DMA queue) ----
            nc.gpsimd.memset(ebase_sb, CBASE)
            nc.vector.memset(v8_sb[:, :, :, 32:33], 1.0)  # ones col of A slice
            nc.vector.memset(ones_sb, 1.0)

            o_ps_a = op.tile([33, 512], F32, tag="o_ps_a")
            o_ps_b = op.tile([32, 512], F32, tag="o_ps_b")

            # The 16 attnV matmuls of a chunk form a WAW chain on the PSUM
            # accumulators, so their emission order IS their execution
            # order. Act-path avs are emitted a few pairs behind their
            # scores (exp latency ~1.3us); GpSimd-path avs are deferred to
            # the chunk end, by which point their pows are long finished.
            DELAY_A = 3
            pend_a = []  # (sc, i, e8) awaiting Act-path attnV
            pend_g = []  # (sc, i, e8) awaiting chunk-end GpSimd-path attnV
            av_seq = {"n": 0}  # position within the chunk's 16-av chain

            def emit_scores(sc, i):
                ssl = slice(sc * 512, (sc + 1) * 512)
                s_ps = sp.tile([P, 2, 512], F32, tag="ps")
                for par in range(2):
                    t = 2 * i + par
                    nc.tensor.matmul(
                        s_ps[:, par, :],
                        r(qk_sb[:, 1, t * P : (t + 1) * P]),
                        r(qk_sb[:, 0, ssl]),
                        start=True, stop=True,
                    )
                e8 = ep.tile([P, 2, 512], F8, tag="e8")
                # chunk 0 is woven into phase B where DVE is busy with
                # projection copies -- keep its exp on Act (Pool idles in B)
                if pattern[i] == "A" or sc == 0:
                    nc.scalar.activation(e8, s_ps, Exp, scale=SCALE)
                    pend_a.append((sc, i, e8))
                else:
                    s_sb = gp.tile([P, 2, 512], F32, tag="s_sb")
                    nc.vector.tensor_copy(s_sb, s_ps)
                    nc.gpsimd.tensor_tensor(e8, ebase_sb, s_sb, Pow)
                    pend_g.append((sc, i, e8))

            def emit_av(entry):
                sc, i, e8 = entry
                first = av_seq["n"] == 0
                last = av_seq["n"] == NPR - 1
                av_seq["n"] = (av_seq["n"] + 1) % NPR
                nc.tensor.matmul(
                    o_ps_a, v8_sb[:, i, :, 0:33], e8,
                    start=first, stop=last, perf_mode=DR,
                )
                nc.tensor.matmul(
                    o_ps_b, v8_sb[:, i, :, 48:80], e8,
                    start=first, stop=last, perf_mode=DR,
                )
                if last:
                    # chunk done: copy O' accumulators out of PSUM. o_lo
                    # gates the epilogue z-transpose -- put it on Act.
                    ssl = slice(sc * 512, (sc + 1) * 512)
                    nc.scalar.copy(r(o_lo[:, ssl]), o_ps_a)
                    nc.vector.tensor_copy(r(o_hi[:, ssl]), o_ps_b)

            def emit_pair(sc, i):
                emit_scores(sc, i)
                if len(pend_a) > DELAY_A:
                    emit_av(pend_a.pop(0))
                if i == NPR - 1:
                    # flush: remaining Act avs, then all GpSimd avs
                    while pend_a:
                        emit_av(pend_a.pop(0))
                    while pend_g:
                        emit_av(pend_g.pop(0))

            # ---- phase B: x^T load + projections, chunk 0 pairs woven in ----
            with tc.tile_pool(name="xtpool", bufs=1) as xtp:
                xt_sb = xtp.tile([P, KT, S], F32, tag="xt")  # x^T (c on parts)
                vt_sb = xtp.tile([HD, S], F32, tag="vt")     # V^T staging
                xt_r = xt.rearrange("(a p) s -> p a s", p=P)

                def xt_load(sc):
                    ssl = slice(sc * 512, (sc + 1) * 512)
                    nc.sync.dma_start(r(xt_sb[:, :, ssl]), r(xt_r[:, :, ssl]))

                # head-critical DMA order: wq, xt0, wk/wv, xt1, rest, xt2..7
                nc.sync.dma_start(r(wq_sb), r(wq.rearrange("(a p) d -> p a d", p=P)))
                xt_load(0)
                nc.sync.dma_start(r(wk_sb), r(wk.rearrange("(a p) d -> p a d", p=P)))
                nc.sync.dma_start(r(wv_sb), r(wv.rearrange("(a p) d -> p a d", p=P)))
                xt_load(1)
                nc.sync.dma_start(r(wpl_sb), r(wp[0:32, :]))
                nc.sync.dma_start(r(wph_sb), r(wp[32:64, :]))
                nc.sync.dma_start(r(id_sb), r(eye))
                for sc in range(2, NSC):
                    xt_load(sc)

                for sc in range(NSC):
                    ssl = slice(sc * 512, (sc + 1) * 512)
                    # Q^T and K^T chunks share one PSUM slot + one copy
                    qk_t = sp.tile([P, 2, 512], F32, tag="ps")
                    for half, w_sb in ((0, wq_sb), (1, wk_sb)):
                        for a in range(KT):
                            nc.tensor.matmul(
                                qk_t[0:HD, half, :],
                                r(w_sb[:, a, :]), r(xt_sb[:, a, ssl]),
                                start=(a == 0), stop=(a == KT - 1),
                            )
                    nc.scalar.copy(r(qk_sb[:, :, ssl]), qk_t[0:HD, :, :])
                    # V^T chunk
                    vt_t = sp.tile([P, 2, 512], F32, tag="ps")
                    for a in range(KT):
                        nc.tensor.matmul(
                            vt_t[0:HD, 0, :], r(wv_sb[:, a, :]),
                            r(xt_sb[:, a, ssl]),
                            start=(a == 0), stop=(a == KT - 1),
                        )
                    nc.vector.tensor_copy(r(vt_sb[:, ssl]), vt_t[0:HD, 0, :])
                    # V natural tiles via PE transpose (4 tiles share one
                    # slot) + two strided fp8-converting copies into v8
                    tp_t = sp.tile([P, 2, 512], F32, tag="ps")
                    for j, t in enumerate(range(4 * sc, 4 * sc + 4)):
                        nc.tensor.transpose(
                            r(tp_t[:, 0, j * HD : (j + 1) * HD]),
                            r(vt_sb[:, t * P : (t + 1) * P]), r(id_sb),
                        )
                    tp_v = tp_t[:, 0, 0:256].rearrange(
                        "p (tl two x) -> p tl two x", tl=2, two=2
                    )  # [P, pair-of-chunk, parity, 64]
                    nc.vector.tensor_copy(
                        v8_sb[:, 2 * sc : 2 * sc + 2, :, 0:32], tp_v[:, :, :, 0:32]
                    )
                    nc.vector.tensor_copy(
                        v8_sb[:, 2 * sc : 2 * sc + 2, :, 48:80], tp_v[:, :, :, 32:64]
                    )
                    # weave chunk-0 pairs (needs kk tiles 4sc..4sc+3 = pairs
                    # 2sc, 2sc+1 and their v8 tiles, all just produced)
                    emit_pair(0, 2 * sc)
                    emit_pair(0, 2 * sc + 1)

            # ---- phase C: chunks 1..7 + deferred epilogues ----
            with tc.tile_pool(name="ystage", bufs=1) as ysp:
                ys_sb = ysp.tile([P, NST, D], F32, tag="ys")

                def epilogue_pe(sc, step):
                    """Chunk sc's epilogue, emitted in 5 steps."""
                    ssl = slice(sc * 512, (sc + 1) * 512)
                    if step == 0:
                        # Z row (o_lo partition 32) -> s-tile-major columns
                        # via K=1 PE matmuls, then 1/Z on DVE. The PSUM slot
                        # is read immediately so it recycles fast.
                        zt_t = sp.tile([P, 2, 512], F32, tag="ps")
                        for j in range(4):
                            st = sc * 4 + j
                            nc.tensor.matmul(
                                zt_t[:, 0, j : j + 1],
                                o_lo[32:33, st * P : (st + 1) * P],
                                ones_sb[32:33, :],
                                start=True, stop=True,
                            )
                        nc.vector.reciprocal(
                            rz_sb[:, sc * 4 : sc * 4 + 4], zt_t[:, 0, 0:4]
                        )
                        return
                    st = sc * 4 + (step - 1)
                    stsl = slice(st * P, (st + 1) * P)
                    y_t = sp.tile([P, 2, 512], F32, tag="ps")
                    y_ps = y_t[:, 0, :]
                    nc.tensor.matmul(
                        y_ps, r(o_lo[0:32, stsl]), r(wpl_sb),
                        start=True, stop=False,
                    )
                    nc.tensor.matmul(
                        y_ps, r(o_hi[0:32, stsl]), r(wph_sb),
                        start=False, stop=True,
                    )
                    if muls[step - 1] == "a":
                        nc.scalar.mul(ys_sb[:, st, :], y_ps, rz_sb[:, st : st + 1])
                    else:
                        nc.vector.tensor_scalar_mul(
                            ys_sb[:, st, :], y_ps, rz_sb[:, st : st + 1]
                        )
                    if step == 4:
                        nc.sync.dma_start(
                            y[ssl, :].rearrange("(st p) d -> p st d", p=P),
                            ys_sb[:, sc * 4 : sc * 4 + 4, :],
                        )

                for sc in range(1, NSC):
                    for i in range(NPR):
                        emit_pair(sc, i)
                        # defer previous chunk's epilogue into this pair
                        # stream (the chunk-close copies are emitted with
                        # the flush at the previous chunk's last pair).
                        if 2 <= i <= 6:
                            epilogue_pe(sc - 1, i - 2)
                # last chunk's epilogue (avs flushed inside emit_pair)
                for step in range(5):
                    epilogue_pe(NSC - 1, step)

    nc.compile()
    return nc


def run(inputs, trace=False, **build_kwargs):
    x = np.asarray(inputs["x"], dtype=np.float32)
    q_param = np.asarray(inputs["q_param"], dtype=np.float32)
    k_param = np.asarray(inputs["k_param"], dtype=np.float32)
    v_param = np.asarray(inputs["v_param"], dtype=np.float32)
    p_param = np.asarray(inputs["p_param"], dtype=np.float32)

    xt = np.ascontiguousarray(x[0].T)  # [D, S]
    eye = np.eye(HD, dtype=np.float32)
    in_maps = []
    for h in range(H):
        in_maps.append(
            {
                "xt": xt,
                "wq": np.ascontiguousarray(q_param[:, h, :]),
                "wk": np.ascontiguousarray(k_param[:, h, :]),
                "wv": np.ascontiguousarray(v_param[:, h, :]),
                "wp": np.ascontiguousarray(p_param[h]),
                "eye": eye,
            }
        )

    nc = build_kernel(**build_kwargs)
    res = run_bass_kernel_spmd(nc, in_maps, core_ids=list(range(H)), trace=trace)
    out = np.zeros((S, D), dtype=np.float32)
    for h in range(H):
        out += res.results[h]["y"]
    return out[None, :, :], res


def kernel(**inputs) -> np.ndarray:
    out, _ = run(inputs, trace=False)
    return out
